# revision 5
# baseline (speedup 1.0000x reference)
"""nn_KeyCompressor Trainium2 kernel (8 NeuronCores, SPMD data-parallel).

Pipeline per token: x/(scale+eps) -> L2 normalize -> LayerNorm -> MLP
(1024 -> 4096 GELU -> 10752 logits) -> per-(group,residual) argmax over 64
codes -> gather codebook centers -> sum residuals -> rescale.

Sharding: 4096 tokens split 512/core across 8 cores; weights replicated.
Heavy matmuls run in float32r (full PE rate at moving-dim >= 256, ~1.5e-4
rel err so argmax flips are rare); the codebook gather is a fp16 matmul
with an exact 0/1 onehot.
"""
import sys

sys.path.insert(0, "/opt/trn_rl_repo")

import numpy as np

import concourse.bass as bass
import concourse.mybir as mybir
import concourse.tile as tile
from concourse.bass_utils import run_bass_kernel_spmd

F32 = mybir.dt.float32
F32R = mybir.dt.float32r
F16 = mybir.dt.float16
AF = mybir.ActivationFunctionType
ALU = mybir.AluOpType
AX = mybir.AxisListType

FEAT = 1024
GS = 128
G = 8
R = 21
KH = 8
K = 64
EPS = 1e-6
LN_EPS = 1e-5
DI = 4096
DE = G * R * K          # 10752
NCORES = 8
TOK = 4096              # 2*2048
TPC = TOK // NCORES     # 512 tokens per core
NTS = TPC // 128        # 4 token-subtiles per core
PAIRS_PER_CHUNK = 7
CHUNK = PAIRS_PER_CHUNK * K   # 448
CPG = R // PAIRS_PER_CHUNK    # 3 chunks per group
DSUB = 2                      # W2 d-subchunks per j-chunk (16 d-tiles each)


def _split_multi_waits(nc):
    # walrus in this container encodes at most ONE sync-wait per
    # instruction; move extra waits onto injected same-engine NoOps.
    nid = 0
    for fn in nc.m.functions:
        for bb in fn.blocks:
            new = []
            for inst in bb.instructions:
                si = inst.sync_info
                ws = list(si.on_wait) if (si is not None and si.on_wait) else []
                if len(ws) > 1:
                    for w in ws[:-1]:
                        nid += 1
                        nop = mybir.InstNoOp(name=f"I-wsplit-{nid}")
                        nop.engine = inst.engine
                        nop.sync_info = mybir.SyncInfo(on_wait=[w], on_update=[])
                        new.append(nop)
                    inst.sync_info = mybir.SyncInfo(
                        on_wait=[ws[-1]], on_update=list(si.on_update or [])
                    )
                new.append(inst)
            bb.instructions = new


def _centers(codebook_params):
    # exact replica of the reference's center construction (fp32)
    T = np.zeros((2 * K, 2 * KH), np.float32)
    for A in range(KH):
        for B in range(KH):
            idx = A * KH + B
            T[2 * idx, 2 * A] = 1.0
            T[2 * idx, 2 * B + 1] = -1.0
            T[2 * idx + 1, 2 * B] = 1.0
            T[2 * idx + 1, 2 * A + 1] = 1.0
    cp = np.asarray(codebook_params, np.float32)
    theta = np.transpose(cp, (0, 1, 2, 4, 3)).reshape(G, R, 2 * KH, GS // 2)
    centers = np.einsum("ij,grjd->grid", T, theta)
    centers = centers.reshape(G, R, K, 2, GS // 2)
    centers = np.transpose(centers, (0, 1, 2, 4, 3)).reshape(G, R, K, GS)
    return centers


def _build(nc, ls_uniform, b2_nonzero):
    dp = nc.declare_dram_parameter
    xin = dp("xin", [TPC, FEAT], F32, isOutput=False)
    w1h = dp("w1h", [FEAT, DI], F16, isOutput=False)
    w1l = dp("w1l", [FEAT, DI], F16, isOutput=False)
    w2h = dp("w2h", [DI, DE], F16, isOutput=False)
    w2l = dp("w2l", [DI, DE], F16, isOutput=False)
    csb = dp("csb", [128, G * CPG * 4 * 128], F16, isOutput=False)
    wbcol = dp("wbcol", [128, 16], F32, isOutput=False)   # [:,0:8]=w  [:,8:16]=b
    scol = dp("scol", [128, G], F32, isOutput=False)       # scale+eps, feature-major
    b1c = dp("b1c", [128, DI // 128], F32, isOutput=False)
    idr = dp("idr", [128, 128], F32R, isOutput=False)
    id16 = dp("id16", [128, 128], F16, isOutput=False)
    id32 = dp("id32", [128, 128], F32, isOutput=False)
    ilsr = None
    if not ls_uniform:
        ilsr = dp("ilsr", [128, FEAT], F32, isOutput=False)
    b2r = None
    if b2_nonzero:
        b2r = dp("b2r", [128, DE], F32, isOutput=False)
    qout = dp("qout", [TPC, FEAT], F32, isOutput=True)
    pout = dp("pout", [TPC, 1], F32, isOutput=True)

    with tile.TileContext(nc) as tc:
        with (
            tc.tile_pool(name="cst", bufs=1) as cst,
            tc.tile_pool(name="psA", bufs=4, space="PSUM") as psA,
            tc.tile_pool(name="psT", bufs=2, space="PSUM") as psT,
            tc.tile_pool(name="psQ", bufs=2, space="PSUM") as psQ,
        ):
            # constants
            idr_t = cst.tile([128, 128], F32R, tag="idr")
            id16_t = cst.tile([128, 128], F16, tag="id16")
            id32_t = cst.tile([128, 128], F32, tag="id32")
            wb_t = cst.tile([128, 16], F32, tag="wb")
            sc_t = cst.tile([128, G], F32, tag="sc")
            b1_t = cst.tile([128, DI // 128], F32, tag="b1")
            nc.sync.dma_start(idr_t[:], idr[:])
            nc.sync.dma_start(id16_t[:], id16[:])
            nc.sync.dma_start(id32_t[:], id32[:])
            nc.sync.dma_start(wb_t[:], wbcol[:])
            nc.sync.dma_start(sc_t[:], scol[:])
            nc.sync.dma_start(b1_t[:], b1c[:])
            ils_t = None
            if not ls_uniform:
                ils_t = cst.tile([128, FEAT], F32, tag="ils")
                nc.sync.dma_start(ils_t[:], ilsr[:])

            pres_all = cst.tile([128, NTS], F32, tag="pres")
            pe_all = cst.tile([128, NTS], F32, tag="pe")
            hTh = cst.tile([128, DI // 128, TPC], F16, tag="hTh")
            hTl = cst.tile([128, DI // 128, TPC], F16, tag="hTl")
            out_ts = [
                cst.tile([128, FEAT], F32, tag=f"out{t}", name=f"out{t}")
                for t in range(NTS)
            ]

            # ---------------- front + mm1 ----------------
            with (
                tc.tile_pool(name="fA", bufs=2) as fA,
                tc.tile_pool(name="sm", bufs=8) as sm,
                tc.tile_pool(name="xnTp", bufs=1) as xnTp,
                tc.tile_pool(name="w1p", bufs=3) as w1p,
            ):
                xnTh = xnTp.tile([128, FEAT // 128, TPC], F16, tag="xnTh")
                xnTl = xnTp.tile([128, FEAT // 128, TPC], F16, tag="xnTl")
                for ts in range(NTS):
                    xt = fA.tile([128, FEAT], F32, tag="xt")
                    nc.sync.dma_start(xt[:], xin[ts * 128:(ts + 1) * 128, :])
                    xs = fA.tile([128, FEAT], F32, tag="xs")
                    if ls_uniform:
                        nc.vector.tensor_scalar(
                            xs[:], xt[:], float(ls_uniform), None, op0=ALU.mult
                        )
                    else:
                        nc.vector.tensor_tensor(
                            xs[:], xt[:], ils_t[:], op=ALU.mult
                        )
                    s0 = sm.tile([128, 1], F32, tag="s0")
                    nc.vector.reduce_sum(s0[:], xs[:], axis=AX.X)
                    sq = fA.tile([128, FEAT], F32, tag="sq")
                    nc.vector.tensor_tensor(sq[:], xs[:], xs[:], op=ALU.mult)
                    s2 = sm.tile([128, 1], F32, tag="s2")
                    nc.vector.reduce_sum(s2[:], sq[:], axis=AX.X)
                    # prescale = sqrt(s2); pe = prescale + EPS; r1 = 1/pe
                    nc.scalar.activation(
                        pres_all[:, ts:ts + 1], s2[:], AF.Sqrt, bias=0.0, scale=1.0
                    )
                    nc.vector.tensor_scalar(
                        pe_all[:, ts:ts + 1], pres_all[:, ts:ts + 1], EPS, None,
                        op0=ALU.add,
                    )
                    r1 = sm.tile([128, 1], F32, tag="r1")
                    nc.vector.reciprocal(r1[:], pe_all[:, ts:ts + 1])
                    # LN stats from s0, s2 (biased var)
                    mu = sm.tile([128, 1], F32, tag="mu")
                    nc.vector.tensor_tensor(mu[:], s0[:], r1[:], op=ALU.mult)
                    nc.vector.tensor_scalar(
                        mu[:], mu[:], 1.0 / FEAT, None, op0=ALU.mult
                    )
                    e2 = sm.tile([128, 1], F32, tag="e2")
                    nc.vector.tensor_tensor(e2[:], s2[:], r1[:], op=ALU.mult)
                    nc.vector.tensor_tensor(e2[:], e2[:], r1[:], op=ALU.mult)
                    nc.vector.tensor_scalar(
                        e2[:], e2[:], 1.0 / FEAT, None, op0=ALU.mult
                    )
                    var = sm.tile([128, 1], F32, tag="var")
                    nc.vector.tensor_tensor(var[:], mu[:], mu[:], op=ALU.mult)
                    nc.vector.tensor_tensor(var[:], e2[:], var[:], op=ALU.subtract)
                    nc.vector.tensor_scalar(
                        var[:], var[:], LN_EPS, None, op0=ALU.add
                    )
                    sd = sm.tile([128, 1], F32, tag="sd")
                    nc.scalar.activation(sd[:], var[:], AF.Sqrt, bias=0.0, scale=1.0)
                    rstd = sm.tile([128, 1], F32, tag="rstd")
                    nc.vector.reciprocal(rstd[:], sd[:])
                    a_ = sm.tile([128, 1], F32, tag="a_")
                    nc.vector.tensor_tensor(a_[:], r1[:], rstd[:], op=ALU.mult)
                    bb_ = sm.tile([128, 1], F32, tag="bb_")
                    nc.vector.tensor_tensor(bb_[:], mu[:], rstd[:], op=ALU.mult)
                    nc.vector.tensor_scalar(
                        bb_[:], bb_[:], -1.0, None, op0=ALU.mult
                    )
                    xnp = fA.tile([128, FEAT], F32, tag="xnp")
                    nc.vector.tensor_scalar(
                        xnp[:], xs[:], a_[:], bb_[:], op0=ALU.mult, op1=ALU.add
                    )
                    # transpose to feature-major; LN affine via per-partition w,b
                    for f in range(FEAT // 128):
                        tr = psT.tile([128, 128], F32, tag="tr")
                        nc.tensor.transpose(
                            tr[:], xnp[:, f * 128:(f + 1) * 128], id32_t[:]
                        )
                        afn = sm.tile([128, 128], F32, tag="afn")
                        nc.vector.tensor_scalar(
                            afn[:], tr[:],
                            wb_t[:, f:f + 1], wb_t[:, 8 + f:9 + f],
                            op0=ALU.mult, op1=ALU.add,
                        )
                        sl = (slice(None), f, slice(ts * 128, (ts + 1) * 128))
                        nc.vector.tensor_copy(xnTh[sl], afn[:])
                        nc.vector.tensor_tensor(
                            xnTl[sl], afn[:], xnTh[sl], op=ALU.subtract
                        )
                # mm1 + GELU -> hT
                for di in range(DI // 128):
                    w1ch = w1p.tile([128, FEAT // 128, 128], F16, tag="w1ch")
                    nc.sync.dma_start(
                        w1ch[:],
                        w1h[:, di * 128:(di + 1) * 128].rearrange(
                            "(ft p) d -> p ft d", p=128
                        ),
                    )
                    w1cl = w1p.tile([128, FEAT // 128, 128], F16, tag="w1cl")
                    nc.sync.dma_start(
                        w1cl[:],
                        w1l[:, di * 128:(di + 1) * 128].rearrange(
                            "(ft p) d -> p ft d", p=128
                        ),
                    )
                    hp = psA.tile([128, TPC], F32, tag="za")
                    nf = FEAT // 128
                    for f in range(nf):
                        nc.tensor.matmul(
                            hp[:], w1ch[:, f, :], xnTh[:, f, :],
                            start=(f == 0), stop=False,
                        )
                        nc.tensor.matmul(
                            hp[:], w1cl[:, f, :], xnTh[:, f, :],
                            start=False, stop=False,
                        )
                        nc.tensor.matmul(
                            hp[:], w1ch[:, f, :], xnTl[:, f, :],
                            start=False, stop=(f == nf - 1),
                        )
                    hf = fA.tile([128, TPC], F32, tag="hf")
                    nc.scalar.activation(
                        hf[:], hp[:], AF.Gelu,
                        bias=b1_t[:, di:di + 1], scale=1.0,
                    )
                    nc.vector.tensor_copy(hTh[:, di, :], hf[:])
                    nc.vector.tensor_tensor(
                        hTl[:, di, :], hf[:], hTh[:, di, :], op=ALU.subtract
                    )

            # ---------------- mm2 + argmax + gather ----------------
            with (
                tc.tile_pool(name="Cp", bufs=1) as Cp,
                tc.tile_pool(name="w2p", bufs=2) as w2p,
                tc.tile_pool(name="ohp", bufs=3) as ohp,
                tc.tile_pool(name="mxp", bufs=8) as mxp,
                tc.tile_pool(name="ohTp", bufs=2) as ohTp,
                tc.tile_pool(name="qsbp", bufs=2) as qsbp,
                tc.tile_pool(name="b2p", bufs=2) as b2p,
            ):
                C_t = Cp.tile([128, G * CPG * 4 * 128], F16, tag="C")
                nc.sync.dma_start(C_t[:], csb[:])
                Cv = C_t[:].rearrange(
                    "p (g c k d) -> p g c k d", g=G, c=CPG, k=4
                )
                for g in range(G):
                    qp = psQ.tile([128, TPC], F32, tag="qp")
                    for c in range(CPG):
                        j0 = (g * CPG + c) * CHUNK
                        w2cs = []
                        for dsub in range(DSUB):
                            r0 = dsub * (DI // DSUB)
                            w2ch = w2p.tile(
                                [128, DI // 128 // DSUB, CHUNK], F16,
                                tag="w2ch", name=f"w2ch{dsub}",
                            )
                            nc.sync.dma_start(
                                w2ch[:],
                                w2h[r0:r0 + DI // DSUB, j0:j0 + CHUNK].rearrange(
                                    "(dt p) j -> p dt j", p=128
                                ),
                            )
                            w2cl = w2p.tile(
                                [128, DI // 128 // DSUB, CHUNK], F16,
                                tag="w2cl", name=f"w2cl{dsub}",
                            )
                            nc.sync.dma_start(
                                w2cl[:],
                                w2l[r0:r0 + DI // DSUB, j0:j0 + CHUNK].rearrange(
                                    "(dt p) j -> p dt j", p=128
                                ),
                            )
                            w2cs.append((w2ch, w2cl))
                        b2t = None
                        if b2_nonzero:
                            b2t = b2p.tile([128, CHUNK], F32, tag="b2t")
                            nc.sync.dma_start(b2t[:], b2r[:, j0:j0 + CHUNK])
                        zts = [
                            psA.tile([128, TPC], F32, tag="za", name=f"zp{ts}")
                            for ts in range(NTS)
                        ]
                        nd = DI // 128 // DSUB
                        for dsub in range(DSUB):
                            w2ch, w2cl = w2cs[dsub]
                            for ts in range(NTS):
                                tsl = slice(ts * 128, (ts + 1) * 128)
                                for dt in range(nd):
                                    d = dsub * nd + dt
                                    first = dsub == 0 and dt == 0
                                    last = dsub == DSUB - 1 and dt == nd - 1
                                    nc.tensor.matmul(
                                        zts[ts][:, :CHUNK],
                                        hTh[:, d, tsl], w2ch[:, dt, :],
                                        start=first, stop=False,
                                    )
                                    nc.tensor.matmul(
                                        zts[ts][:, :CHUNK],
                                        hTh[:, d, tsl], w2cl[:, dt, :],
                                        start=False, stop=False,
                                    )
                                    nc.tensor.matmul(
                                        zts[ts][:, :CHUNK],
                                        hTl[:, d, tsl], w2ch[:, dt, :],
                                        start=False, stop=last,
                                    )
                        ohT = ohTp.tile([128, 4, TPC], F16, tag="ohT")
                        for ts in range(NTS):
                            zp = zts[ts]
                            if b2_nonzero:
                                nc.vector.tensor_tensor(
                                    zp[:, :CHUNK], zp[:, :CHUNK], b2t[:],
                                    op=ALU.add,
                                )
                            mx = mxp.tile([128, PAIRS_PER_CHUNK], F32, tag="mx")
                            nc.vector.tensor_reduce(
                                mx[:],
                                zp[:, :CHUNK].rearrange(
                                    "p (n k) -> p n k", k=K
                                ),
                                axis=AX.X, op=ALU.max,
                            )
                            oh = ohp.tile([128, 512], F16, tag="oh")
                            nc.vector.tensor_tensor(
                                oh[:, :CHUNK].rearrange(
                                    "p (n k) -> p n k", k=K
                                ),
                                zp[:, :CHUNK].rearrange(
                                    "p (n k) -> p n k", k=K
                                ),
                                mx[:].unsqueeze(2).broadcast_to(
                                    [128, PAIRS_PER_CHUNK, K]
                                ),
                                op=ALU.is_ge,
                            )
                            nc.vector.memset(oh[:, CHUNK:], 0.0)
                            for kk in range(4):
                                trm = psT.tile([128, 128], F16, tag="tr")
                                nc.tensor.transpose(
                                    trm[:], oh[:, kk * 128:(kk + 1) * 128],
                                    id16_t[:],
                                )
                                nc.scalar.copy(
                                    ohT[:, kk, ts * 128:(ts + 1) * 128], trm[:]
                                )
                        for kk in range(4):
                            nc.tensor.matmul(
                                qp[:], Cv[:, g, c, kk, :], ohT[:, kk, :],
                                start=(c == 0 and kk == 0),
                                stop=(c == CPG - 1 and kk == 3),
                            )
                    # epilogue for group g
                    qsb = qsbp.tile([128, TPC], F32, tag="qsb")
                    nc.scalar.activation(
                        qsb[:], qp[:], AF.Copy, bias=0.0, scale=sc_t[:, g:g + 1]
                    )
                    for ts in range(NTS):
                        trq = psT.tile([128, 128], F32, tag="tr")
                        nc.tensor.transpose(
                            trq[:], qsb[:, ts * 128:(ts + 1) * 128], id32_t[:]
                        )
                        nc.vector.tensor_scalar(
                            out_ts[ts][:, g * 128:(g + 1) * 128], trq[:],
                            pe_all[:, ts:ts + 1], None, op0=ALU.mult,
                        )
            for ts in range(NTS):
                nc.sync.dma_start(
                    qout[ts * 128:(ts + 1) * 128, :], out_ts[ts][:]
                )
            nc.sync.dma_start(
                pout.rearrange("(t p) o -> p (t o)", p=128), pres_all[:]
            )
    _split_multi_waits(nc)
    return nc


_CACHE = {}


def kernel(x, learnable_scale, ln_weight, ln_bias, W1, b1, W2, b2,
           codebook_params):
    x = np.asarray(x, np.float32)
    ls = np.asarray(learnable_scale, np.float32)
    lw = np.asarray(ln_weight, np.float32)
    lb = np.asarray(ln_bias, np.float32)
    W1 = np.asarray(W1, np.float32)
    b1 = np.asarray(b1, np.float32)
    W2 = np.asarray(W2, np.float32)
    b2 = np.asarray(b2, np.float32)

    ls_uniform = 0.0
    if np.ptp(ls) == 0.0:
        ls_uniform = float(1.0 / (ls.flat[0] + EPS))
    b2_nonzero = bool(np.any(b2 != 0.0))

    key = (bool(ls_uniform), b2_nonzero)
    if key not in _CACHE:
        nc = bass.Bass()
        _build(nc, ls_uniform, b2_nonzero)
        _CACHE[key] = nc
    nc = _CACHE[key]

    centers = _centers(codebook_params)
    # fp16 center table [128, G, CPG, 4, 128]: per (group, chunk) 4 row-tiles
    # of the chunk's 448 js (4th tile half, zero-padded)
    Chost = np.zeros((128, G, CPG, 4, 128), np.float16)
    for g in range(G):
        CG = centers[g].reshape(R * K, GS)
        for c in range(CPG):
            for kk in range(4):
                lo = c * CHUNK + kk * 128
                n = min(128, c * CHUNK + CHUNK - lo)
                if n > 0:
                    Chost[:n, g, c, kk, :] = CG[lo:lo + n, :]
    Chost = np.ascontiguousarray(Chost.reshape(128, G * CPG * 4 * 128))

    w1t = np.ascontiguousarray(W1.T)                 # [1024, 4096]
    w2t = np.ascontiguousarray(W2.T)                 # [4096, 10752]
    w1hh = w1t.astype(np.float16)
    w1ll = (w1t - w1hh.astype(np.float32)).astype(np.float16)
    w2hh = w2t.astype(np.float16)
    w2ll = (w2t - w2hh.astype(np.float32)).astype(np.float16)
    wbcol = np.concatenate(
        [lw.reshape(G, 128).T, lb.reshape(G, 128).T], axis=1
    ).astype(np.float32)                             # [128, 16]
    scol = np.ascontiguousarray((ls + EPS).reshape(G, 128).T)
    b1c = np.ascontiguousarray(b1.reshape(DI // 128, 128).T)
    idr = np.eye(128, dtype=np.float32)
    id16 = np.eye(128, dtype=np.float16)

    xflat = np.ascontiguousarray(x.reshape(TOK, FEAT))
    in_maps = []
    for c in range(NCORES):
        m = {
            "xin": np.ascontiguousarray(xflat[c * TPC:(c + 1) * TPC]),
            "w1h": w1hh, "w1l": w1ll, "w2h": w2hh, "w2l": w2ll,
            "csb": Chost, "wbcol": wbcol,
            "scol": scol, "b1c": b1c, "idr": idr, "id16": id16, "id32": idr,
        }
        if not ls_uniform:
            m["ilsr"] = np.broadcast_to(
                (1.0 / (ls + EPS)).astype(np.float32), (128, FEAT)
            ).copy()
        if b2_nonzero:
            m["b2r"] = np.broadcast_to(b2.astype(np.float32), (128, DE)).copy()
        in_maps.append(m)

    res = run_bass_kernel_spmd(nc, in_maps, list(range(NCORES)))
    quant = np.concatenate(
        [res.results[c]["qout"] for c in range(NCORES)], axis=0
    ).reshape(2, 2048, FEAT)
    prescale = np.concatenate(
        [res.results[c]["pout"] for c in range(NCORES)], axis=0
    ).reshape(2, 2048, 1)
    return quant, prescale, np.float32(0.0)
